# revision 17
# baseline (speedup 1.0000x reference)
"""Trainium2 Bass kernel for BasicMultiTokenPrediction.

Strategy (8 NeuronCores, SPMD, two launches):
  Stage A — the 3 sequential transformer layers, data-parallel over the 506
  independent (batch, position) rows: 64 rows per core, activations kept
  transposed in SBUF ([feature-on-partition, row-on-free]).  The attention
  V-projection and out-projection are folded on the host into a single
  [D, D] matrix (softmax over one key is 1), removing one GEMM per layer.
  The sa/ff1/ff2 weights are stored in fp8-e3m4 (x128 scale, exact since
  128 = 2^7; the matching 1/128 is folded into the producing op of each
  GEMM's bf16 input so PSUM comes out at true scale); proj stays bf16.
  This cuts the replicated per-core weight DMA from 48MB to ~28MB, which
  is the stage-A bottleneck (HBM-per-core ~358 GB/s).  Weight tiles are
  split into two output-halves so MMs start when half a weight has landed.
  Stage B — the dominant vocab unembed GEMM, tensor-parallel over embed
  rows: each core takes a 6400-wide vocab shard (V padded 50257→51200) and
  all 3*506 hidden rows. lhsT = hidden states, rhs = embed shard streamed
  through SBUF; 8 accumulating matmuls per 128x512 PSUM tile.  PSUM is
  drained to bf16 staging tiles (2x DVE/ACT copy throughput, half the
  store DMA) and written to DRAM in ~1.5MB chunks; the host upcasts.

Host side: shards/transposes/casts inputs, gathers h between launches,
concatenates vocab shards at the end.  Zero biases / unit norm scales skip
their device ops entirely (general path emitted only when needed; rms_w is
always folded into the proj weights on the host, which is exact).
"""
import sys

try:
    import concourse.bass  # noqa: F401  (already on path in this env?)
except ImportError:
    sys.path.insert(0, "/opt/trn_rl_repo")

import numpy as np
import ml_dtypes
import concourse.bass as bass
import concourse.mybir as mybir
import concourse.tile as tile
from concourse import bacc
from concourse import bass2jax

BF16 = mybir.dt.bfloat16
F32 = mybir.dt.float32
F8 = mybir.dt.float8e3            # e3m4
BF16_NP = ml_dtypes.bfloat16
F8_NP = ml_dtypes.float8_e3m4
F8S = 128.0                       # fp8 weight scale (2^7, exact)
F8MAX = 15.5

B, T, D, K, V, DFF = 2, 256, 1024, 3, 50257, 2048
P = T - K            # 253 starting positions
ROWS = B * P         # 506 independent rows
NCORES = 8
VS = 6400            # per-core vocab shard (8*6400 = 51200 >= 50257)
VPAD = VS * NCORES
DC = D // 128        # 8 contraction chunks of 128
RPC = 64             # stage-A rows per core (8*64 = 512 >= 506)
FC_PROJ = (2 * D) // 128
OC = D // 128
EPS_LN = 1e-5
EPS_RMS = 1e-6

# which stage-A weights are stored fp8-e3m4 (proj must stay bf16)
A_WDT = {"wproj": "bf16", "wsa": "f8", "wf1": "f8", "wf2": "f8"}
OUT_BF16 = True      # stage-B logits staged+stored as bf16, host upcasts

ROWS_B = 512         # stage-B rows padded to 4 full 128-row tiles
NRT = ROWS_B // 128
V_BLOCKS = [(j * 512, 512) for j in range(VS // 512)]
if VS % 512:
    V_BLOCKS.append((VS - VS % 512, VS % 512))

RELU = mybir.ActivationFunctionType.Relu


# =============================================================== stage A ====

def build_stage_a(flags, repeat=1, ab_mode=None):
    """flags: dict of bools — pbias, vbias, aobias, f1bias, f2bias, ln1, ln2.
    Weights arrive pre-tiled [K, 2, 128, FC, O/2] (output-dim halves) so
    every DMA reads a fully contiguous per-partition block.
    ab_mode: None | 'dma_only' (loads only) | 'mm_only' (no weight DMA)."""
    nc = bacc.Bacc("TRN2", target_bir_lowering=False, debug=False,
                   num_devices=NCORES)
    sa_f8 = A_WDT["wsa"] == "f8"
    f1_f8 = A_WDT["wf1"] == "f8"
    f2_f8 = A_WDT["wf2"] == "f8"
    assert A_WDT["wproj"] == "bf16"
    wdt = {k: (F8 if v == "f8" else BF16) for k, v in A_WDT.items()}

    mh0 = nc.dram_tensor("mh0", [128, DC, RPC], BF16, kind="ExternalInput").ap()
    me = nc.dram_tensor("me", [K, 128, DC, RPC], BF16, kind="ExternalInput").ap()
    wproj = nc.dram_tensor("wproj", [K, 2, 128, FC_PROJ, D // 2],
                           wdt["wproj"], kind="ExternalInput").ap()
    wsa = nc.dram_tensor("wsa", [K, 2, 128, DC, D // 2], wdt["wsa"],
                         kind="ExternalInput").ap()
    wf1 = nc.dram_tensor("wf1", [K, 2, 128, DC, DFF // 2], wdt["wf1"],
                         kind="ExternalInput").ap()
    wf2 = nc.dram_tensor("wf2", [K, 2, 128, FC_PROJ, D // 2], wdt["wf2"],
                         kind="ExternalInput").ap()
    hTo = nc.dram_tensor("hTo", [K, DC, 128, RPC], BF16,
                         kind="ExternalOutput").ap()

    bias_aps = {}
    for nm, width in (("pbias", D), ("vbias", D), ("aobias", D),
                      ("f1bias", DFF), ("f2bias", D)):
        if flags.get(nm):
            bias_aps[nm] = nc.dram_tensor(
                nm, [K, width], F32, kind="ExternalInput").ap()
    for nm in ("ln1", "ln2"):
        if flags.get(nm):
            bias_aps[nm + "w"] = nc.dram_tensor(
                nm + "w", [K, D], F32, kind="ExternalInput").ap()
            bias_aps[nm + "b"] = nc.dram_tensor(
                nm + "b", [K, D], F32, kind="ExternalInput").ap()

    with tile.TileContext(nc) as tc:
        with (
            tc.tile_pool(name="singles", bufs=1) as singles,
            tc.tile_pool(name="wpool", bufs=3) as wpool,
            tc.tile_pool(name="bpool", bufs=2) as bpool,
            tc.tile_pool(name="acts", bufs=2) as acts,
            tc.tile_pool(name="stats", bufs=4) as stats,
            tc.tile_pool(name="gp", bufs=4, space="PSUM") as gp,
            tc.tile_pool(name="sp", bufs=2, space="PSUM") as sp,
            tc.tile_pool(name="bp", bufs=2, space="PSUM") as bp,
        ):
            ones128 = singles.tile([128, 1], F32, name="ones128")
            nc.vector.memset(ones128, 1.0)
            ones1 = singles.tile([1, 128], F32, name="ones1")
            nc.vector.memset(ones1, 1.0)
            eps_ln = singles.tile([1, 1], F32, name="eps_ln")
            nc.vector.memset(eps_ln, EPS_LN)
            eps_rms = singles.tile([1, 1], F32, name="eps_rms")
            nc.vector.memset(eps_rms, EPS_RMS)

            w_ring = [0]

            def load_w(wdram, k, fc, oh, nm):
                halves = []
                for hi in range(2):
                    t = wpool.tile([128, fc, oh], wdt[nm], tag=f"w_{nm}",
                                   name=f"{nm}_{k}_{hi}")
                    # alternate loads between the two HW-DGE rings so the
                    # ~2µs completion tails overlap
                    if ab_mode != "mm_only":
                        eng = nc.sync if w_ring[0] % 2 == 0 else nc.scalar
                        w_ring[0] += 1
                        eng.dma_start(out=t, in_=wdram[k, hi])
                    halves.append(t)
                return halves

            def load_bias(nm, k, width):
                t = bpool.tile([128, width // 128], F32, tag=f"b_{nm}",
                               name=f"{nm}_{k}")
                nc.sync.dma_start(out=t, in_=bias_aps[nm][k].rearrange(
                    "(c p) -> p c", p=128))
                return t

            def gemm(halves, fc, oh, rhs, consume, nm, k):
                otn = oh // 128
                for hi, Wsb in enumerate(halves):
                    for ot in range(otn):
                        ps = gp.tile([128, RPC], F32, tag="gp",
                                     name=f"ps_{nm}_{k}_{hi}_{ot}")
                        for f in range(fc):
                            nc.tensor.matmul(
                                ps, lhsT=Wsb[:, f, ot * 128:(ot + 1) * 128],
                                rhs=rhs[:, f, :], start=(f == 0),
                                stop=(f == fc - 1))
                        consume(hi * otn + ot, ps)

            def colsum(src, nm, k):
                ps = sp.tile([1, RPC], F32, tag="sp", name=f"ss_{nm}_{k}")
                for c in range(DC):
                    nc.tensor.matmul(ps, lhsT=ones128, rhs=src[:, c, :],
                                     start=(c == 0), stop=(c == DC - 1))
                return ps

            def bcast(vec, nm, k):
                ps = bp.tile([128, RPC], F32, tag="bp", name=f"bc_{nm}_{k}")
                nc.tensor.matmul(ps, lhsT=ones1, rhs=vec, start=True, stop=True)
                return ps

            def rstd_from_ex2(ex2, eps_tile, nm, k):
                sd = stats.tile([1, RPC], F32, tag="sd", name=f"sd_{nm}_{k}")
                nc.scalar.activation(out=sd, in_=ex2,
                                     func=mybir.ActivationFunctionType.Sqrt,
                                     bias=eps_tile, scale=1.0)
                rs = stats.tile([1, RPC], F32, tag="rs", name=f"rs_{nm}_{k}")
                nc.vector.reciprocal(out=rs, in_=sd)
                return rs

            def layernorm(src, outs, nm, k, affine=None):
                """outs: list of (tile [128, DC, RPC], scale); first is the
                fp32 primary (scale must be 1), extras written from it on
                the scalar engine with their scale folded in."""
                prim, pscale = outs[0]
                assert pscale == 1.0
                sq = acts.tile([128, DC, RPC], F32, tag="sq", name=f"sq_{nm}_{k}")
                for c in range(DC):
                    nc.vector.tensor_mul(out=sq[:, c, :], in0=src[:, c, :],
                                         in1=src[:, c, :])
                ps_sum = colsum(src, f"{nm}s", k)
                ps_ss = colsum(sq, f"{nm}q", k)
                m1 = stats.tile([1, RPC], F32, tag="m1", name=f"m1_{nm}_{k}")
                nc.scalar.mul(out=m1, in_=ps_sum, mul=1.0 / D)
                e1 = stats.tile([1, RPC], F32, tag="e1", name=f"e1_{nm}_{k}")
                nc.scalar.mul(out=e1, in_=ps_ss, mul=1.0 / D)
                msq = stats.tile([1, RPC], F32, tag="msq", name=f"msq_{nm}_{k}")
                nc.vector.tensor_mul(out=msq, in0=m1, in1=m1)
                var = stats.tile([1, RPC], F32, tag="var", name=f"var_{nm}_{k}")
                nc.vector.tensor_sub(out=var, in0=e1, in1=msq)
                rs = rstd_from_ex2(var, eps_ln, nm, k)
                mb_ps = bcast(m1, f"{nm}m", k)
                rb_ps = bcast(rs, f"{nm}r", k)
                mb = acts.tile([128, RPC], F32, tag="mb", name=f"mb_{nm}_{k}")
                nc.vector.tensor_copy(out=mb, in_=mb_ps)
                rb = acts.tile([128, RPC], F32, tag="rb", name=f"rb_{nm}_{k}")
                nc.vector.tensor_copy(out=rb, in_=rb_ps)
                for c in range(DC):
                    cen = sq[:, c, :]
                    nc.vector.tensor_sub(out=cen, in0=src[:, c, :], in1=mb)
                    if affine is None:
                        nc.vector.tensor_mul(out=prim[:, c, :], in0=cen, in1=rb)
                    else:
                        w_t, b_t = affine
                        nc.vector.tensor_mul(out=cen, in0=cen, in1=rb)
                        nc.vector.tensor_scalar(
                            out=prim[:, c, :], in0=cen,
                            scalar1=w_t[:, c:c + 1], scalar2=b_t[:, c:c + 1],
                            op0=mybir.AluOpType.mult, op1=mybir.AluOpType.add)
                    for out_t, s in outs[1:]:
                        if s == 1.0:
                            nc.scalar.copy(out=out_t[:, c, :],
                                           in_=prim[:, c, :])
                        else:
                            nc.scalar.mul(out=out_t[:, c, :],
                                          in_=prim[:, c, :], mul=s)

            def ln_center(src, outs, nm, k):
                """Subtract the per-row feature mean only (no rstd).  Exact
                when ln1's affine is identity: the skipped per-row scale is
                positive and cancels through relu/ff GEMMs into ln2, which
                is scale-invariant."""
                prim, pscale = outs[0]
                assert pscale == 1.0
                ps_sum = colsum(src, f"{nm}s", k)
                m1 = stats.tile([1, RPC], F32, tag="m1", name=f"m1_{nm}_{k}")
                nc.scalar.mul(out=m1, in_=ps_sum, mul=1.0 / D)
                mb_ps = bcast(m1, f"{nm}m", k)
                mb = acts.tile([128, RPC], F32, tag="mb", name=f"mb_{nm}_{k}")
                nc.vector.tensor_copy(out=mb, in_=mb_ps)
                for c in range(DC):
                    nc.vector.tensor_sub(out=prim[:, c, :], in0=src[:, c, :],
                                         in1=mb)
                    for out_t, s in outs[1:]:
                        if s == 1.0:
                            nc.scalar.copy(out=out_t[:, c, :],
                                           in_=prim[:, c, :])
                        else:
                            nc.scalar.mul(out=out_t[:, c, :],
                                          in_=prim[:, c, :], mul=s)

            # mrg kept as two tiles: the e-half arrives by DMA and has no
            # dependency on the previous layer, so its proj MMs can fill the
            # PE while the h-half's rms/ln2 stat chains are in flight.
            mrgh0 = acts.tile([128, DC, RPC], BF16, tag="mrgh", name="mrgh_0")
            mrge0 = acts.tile([128, DC, RPC], BF16, tag="mrge", name="mrge_0")
            # ACT ring: keep the small input loads out of the SP ring's FIFO
            # so the first proj weight half starts streaming immediately
            nc.scalar.dma_start(out=mrgh0, in_=mh0)
            nc.scalar.dma_start(out=mrge0, in_=me[0])

            def a_body(iv=None):
                mrgh_k, mrge_k = mrgh0, mrge0
                for k in range(K):
                    Wp = load_w(wproj, k, FC_PROJ, D // 2, "wproj")
                    Ws = load_w(wsa, k, DC, D // 2, "wsa")
                    W1 = load_w(wf1, k, DC, DFF // 2, "wf1")
                    W2 = load_w(wf2, k, FC_PROJ, D // 2, "wf2")
                    if ab_mode == "dma_only":
                        continue
                    pb = load_bias("pbias", k, D) if flags.get("pbias") else None
                    ab = load_bias("aobias", k, D) if flags.get("aobias") else None
                    f1b = load_bias("f1bias", k, DFF) if flags.get("f1bias") else None
                    f2b = load_bias("f2bias", k, D) if flags.get("f2bias") else None
                    ln1a = ((load_bias("ln1w", k, D), load_bias("ln1b", k, D))
                            if flags.get("ln1") else None)
                    ln2a = ((load_bias("ln2w", k, D), load_bias("ln2b", k, D))
                            if flags.get("ln2") else None)

                    p_f32 = acts.tile([128, DC, RPC], F32, tag="p_f32",
                                      name=f"p_f32_{k}")
                    p_bf = acts.tile([128, DC, RPC], BF16, tag="p_bf",
                                     name=f"p_bf_{k}")

                    def c_proj(ot, ps, p_f32=p_f32, p_bf=p_bf, pb=pb):
                        if pb is None:
                            nc.vector.tensor_copy(out=p_f32[:, ot, :], in_=ps)
                        else:
                            nc.vector.tensor_scalar_add(
                                out=p_f32[:, ot, :], in0=ps,
                                scalar1=pb[:, ot:ot + 1])
                        if sa_f8:
                            nc.scalar.mul(out=p_bf[:, ot, :],
                                          in_=p_f32[:, ot, :], mul=1.0 / F8S)
                        else:
                            nc.vector.tensor_copy(out=p_bf[:, ot, :],
                                                  in_=p_f32[:, ot, :])

                    # e-part MMs first (DMA-only dependency), h-part second
                    otn = (D // 2) // 128
                    for hi, Wsb in enumerate(Wp):
                        for ot in range(otn):
                            ps = gp.tile([128, RPC], F32, tag="gp",
                                         name=f"ps_proj_{k}_{hi}_{ot}")
                            for f in range(DC, FC_PROJ):
                                nc.tensor.matmul(
                                    ps, lhsT=Wsb[:, f, ot * 128:(ot + 1) * 128],
                                    rhs=mrge_k[:, f - DC, :],
                                    start=(f == DC), stop=False)
                            for f in range(DC):
                                nc.tensor.matmul(
                                    ps, lhsT=Wsb[:, f, ot * 128:(ot + 1) * 128],
                                    rhs=mrgh_k[:, f, :],
                                    start=False, stop=(f == DC - 1))
                            c_proj(hi * otn + ot, ps)

                    r1 = acts.tile([128, DC, RPC], F32, tag="r1", name=f"r1_{k}")

                    def c_sa(ot, ps, r1=r1, p_f32=p_f32, ab=ab):
                        nc.vector.tensor_add(out=r1[:, ot, :], in0=ps,
                                             in1=p_f32[:, ot, :])
                        if ab is not None:
                            nc.vector.tensor_scalar_add(
                                out=r1[:, ot, :], in0=r1[:, ot, :],
                                scalar1=ab[:, ot:ot + 1])
                    gemm(Ws, DC, D // 2, p_bf, c_sa, "sa", k)

                    s1_f32 = acts.tile([128, DC, RPC], F32, tag="s1_f32",
                                       name=f"s1_f32_{k}")
                    s1_bf = acts.tile([128, DC, RPC], BF16, tag="s1_bf",
                                      name=f"s1_bf_{k}")
                    s1_outs = [(s1_f32, 1.0),
                               (s1_bf, (1.0 / F8S) if f1_f8 else 1.0)]
                    if ln1a is None:
                        ln_center(r1, s1_outs, "ln1", k)
                    else:
                        layernorm(r1, s1_outs, "ln1", k, affine=ln1a)

                    ff_bf = acts.tile([128, FC_PROJ, RPC], BF16, tag="ff_bf",
                                      name=f"ff_bf_{k}")
                    relu_scale = (1.0 / F8S) if f2_f8 else 1.0

                    def c_ff1(ot, ps, ff_bf=ff_bf, f1b=f1b, rs=relu_scale):
                        if f1b is None and rs == 1.0:
                            nc.vector.tensor_relu(out=ff_bf[:, ot, :], in_=ps)
                        else:
                            # out = relu(ps*rs + f1b*rs); host pre-scales f1b
                            nc.scalar.activation(
                                out=ff_bf[:, ot, :], in_=ps, func=RELU,
                                bias=(0.0 if f1b is None
                                      else f1b[:, ot:ot + 1]),
                                scale=rs)
                    gemm(W1, DC, DFF // 2, s1_bf, c_ff1, "ff1", k)

                    r2 = acts.tile([128, DC, RPC], F32, tag="r2", name=f"r2_{k}")

                    def c_ff2(ot, ps, r2=r2, s1_f32=s1_f32, f2b=f2b):
                        nc.vector.tensor_add(out=r2[:, ot, :], in0=ps,
                                             in1=s1_f32[:, ot, :])
                        if f2b is not None:
                            nc.vector.tensor_scalar_add(
                                out=r2[:, ot, :], in0=r2[:, ot, :],
                                scalar1=f2b[:, ot:ot + 1])
                    gemm(W2, FC_PROJ, D // 2, ff_bf, c_ff2, "ff2", k)

                    h_bf = acts.tile([128, DC, RPC], BF16, tag="h_bf",
                                     name=f"h_bf_{k}")
                    layernorm(r2, [(h_bf, 1.0)], "ln2", k, affine=ln2a)

                    nc.sync.dma_start(
                        out=hTo[k].rearrange("c p r -> p c r"), in_=h_bf)

                    if k < K - 1:
                        mrgh_k = acts.tile([128, DC, RPC], BF16, tag="mrgh",
                                           name=f"mrgh_{k + 1}")
                        mrge_k = acts.tile([128, DC, RPC], BF16, tag="mrge",
                                           name=f"mrge_{k + 1}")
                        nc.sync.dma_start(out=mrge_k, in_=me[k + 1])
                        sqh = acts.tile([128, DC, RPC], F32, tag="sq",
                                        name=f"sqh_{k}")
                        for c in range(DC):
                            nc.vector.tensor_mul(out=sqh[:, c, :],
                                                 in0=h_bf[:, c, :],
                                                 in1=h_bf[:, c, :])
                        ps_ss = colsum(sqh, "rms", k)
                        e1 = stats.tile([1, RPC], F32, tag="e1",
                                        name=f"e1_rms_{k}")
                        nc.scalar.mul(out=e1, in_=ps_ss, mul=1.0 / D)
                        rs = rstd_from_ex2(e1, eps_rms, "rms", k)
                        rb_ps = bcast(rs, "rms", k)
                        rb = acts.tile([128, RPC], F32, tag="rb",
                                       name=f"rb_rms_{k}")
                        nc.vector.tensor_copy(out=rb, in_=rb_ps)
                        for c in range(DC):
                            nc.vector.tensor_mul(out=mrgh_k[:, c, :],
                                                 in0=h_bf[:, c, :], in1=rb)

            if repeat > 1:
                with tc.For_i(0, repeat, 1):
                    a_body()
            else:
                a_body()
    nc.compile()
    return nc


# =============================================================== stage B ====

NBLK = len(V_BLOCKS)
OUT_DT = BF16 if OUT_BF16 else F32
OUT_DT_NP = BF16_NP if OUT_BF16 else np.float32


def build_stage_b(repeat=1, mm_only=False, no_dma=False, out_split=True,
                  psum_bufs=2, spool_bufs=3, copy_eng="split",
                  ring_swap=False):
    """Output is written block-major ([NBLK, 128, K, NRT, 512]) so every
    store is one fully-contiguous DMA; the host untangles the layout.
    PSUM tiles span 4 banks (all 4 row-tiles of one k) so bank-handoff
    semaphores and drain copies happen 4x less often."""
    nc = bacc.Bacc("TRN2", target_bir_lowering=False, debug=False,
                   num_devices=NCORES)
    hT = nc.dram_tensor("hT", [K, D, ROWS_B], BF16, kind="ExternalInput").ap()
    emb = nc.dram_tensor("emb", [128, NBLK, DC, 512], BF16,
                         kind="ExternalInput").ap()
    out = nc.dram_tensor("out", [NBLK, 128, K, NRT, 512], OUT_DT,
                         kind="ExternalOutput").ap()

    with tile.TileContext(nc) as tc:
        with (
            tc.tile_pool(name="hpool", bufs=1) as hpool,
            tc.tile_pool(name="epool", bufs=4) as epool,
            tc.tile_pool(name="spool", bufs=spool_bufs) as spool,
            tc.tile_pool(name="psum", bufs=psum_bufs, space="PSUM") as pp,
        ):
            hsb = []
            for k in range(K):
                row = []
                for c in range(DC):
                    t = hpool.tile([128, ROWS_B], BF16, name=f"h_{k}_{c}")
                    # ACT ring: the embed stream owns the SP ring, so the h
                    # preload doesn't delay the first v-block
                    nc.scalar.dma_start(out=t,
                                        in_=hT[k, c * 128:(c + 1) * 128, :])
                    row.append(t)
                hsb.append(row)

            def b_body(iv=None):
                for j, (v0, vn) in enumerate(V_BLOCKS):
                    ej = epool.tile([128, DC, 512], BF16, name="ej", tag="ej")
                    (nc.scalar if ring_swap else nc.sync).dma_start(
                        out=ej, in_=emb[:, j])
                    stb = spool.tile([128, K, NRT, 512], OUT_DT, name="stb",
                                     tag="st")
                    for k in range(K):
                        ps = pp.tile([128, NRT, 512], F32, name="ps", tag="ps")
                        for rt in range(NRT):
                            for c in range(DC):
                                nc.tensor.matmul(
                                    ps[:, rt, :vn],
                                    lhsT=hsb[k][c][:, rt * 128:(rt + 1) * 128],
                                    rhs=ej[:, c, :vn],
                                    start=(c == 0), stop=(c == DC - 1))
                        if not mm_only:
                            # one 4-bank drain per k; alternate DVE/ACT
                            use_dve = (copy_eng == "dve" or
                                       (copy_eng == "split" and k % 2 == 0))
                            if use_dve:
                                nc.vector.tensor_copy(
                                    out=stb[:, k, :, :vn],
                                    in_=ps[:, :, :vn])
                            else:
                                nc.scalar.copy(out=stb[:, k, :, :vn],
                                               in_=ps[:, :, :vn])
                    if mm_only or no_dma:
                        continue
                    if ring_swap:
                        eng = nc.sync
                    else:
                        eng = nc.sync if (out_split and j % 2) else nc.scalar
                    eng.dma_start(out=out[j], in_=stb)

            if repeat > 1:
                with tc.For_i(0, repeat, 1):
                    b_body()
            else:
                b_body()
    nc.compile()
    return nc


# ================================================================ runner ====

def make_runner(nc, n_cores=NCORES):
    import jax
    from jax.sharding import Mesh, PartitionSpec
    from jax.experimental.shard_map import shard_map

    bass2jax.install_neuronx_cc_hook()
    partition_name = nc.partition_id_tensor.name if nc.partition_id_tensor else None
    in_names, out_names, out_avals, zero_outs = [], [], [], []
    for alloc in nc.m.functions[0].allocations:
        if not isinstance(alloc, mybir.MemoryLocationSet):
            continue
        name = alloc.memorylocations[0].name
        if alloc.kind == "ExternalInput":
            if name != partition_name:
                in_names.append(name)
        elif alloc.kind == "ExternalOutput":
            out_names.append(name)
            shape = tuple(alloc.tensor_shape)
            dtype = mybir.dt.np(alloc.dtype)
            out_avals.append(jax.core.ShapedArray(shape, dtype))
            zero_outs.append(np.zeros(shape, dtype))
    n_params = len(in_names)
    in_names_all = in_names + out_names
    if partition_name is not None:
        in_names_all.append(partition_name)

    def _body(*args):
        operands = list(args)
        if partition_name is not None:
            operands.append(bass2jax.partition_id_tensor())
        outs = bass2jax._bass_exec_p.bind(
            *operands, out_avals=tuple(out_avals), in_names=tuple(in_names_all),
            out_names=tuple(out_names), lowering_input_output_aliases=(),
            sim_require_finite=True, sim_require_nnan=True, nc=nc)
        return tuple(outs)

    devices = jax.devices()[:n_cores]
    mesh = Mesh(np.asarray(devices), ("core",))
    in_specs = (PartitionSpec("core"),) * (n_params + len(out_names))
    out_specs = (PartitionSpec("core"),) * len(out_names)
    fn = jax.jit(
        shard_map(_body, mesh=mesh, in_specs=in_specs, out_specs=out_specs,
                  check_rep=False),
        keep_unused=True)

    def stage(in_maps):
        from jax.sharding import NamedSharding
        sh = NamedSharding(mesh, PartitionSpec("core"))
        concat_in = [np.concatenate([np.asarray(m[name]) for m in in_maps],
                                    axis=0) for name in in_names]
        concat_zeros = [np.zeros((n_cores * z.shape[0], *z.shape[1:]), z.dtype)
                        for z in zero_outs]
        return [jax.device_put(a, sh) for a in concat_in + concat_zeros]

    def run_staged(args):
        out_arrs = fn(*args)
        return [
            {name: np.asarray(out_arrs[i]).reshape(n_cores, *out_avals[i].shape)[c]
             for i, name in enumerate(out_names)}
            for c in range(n_cores)
        ]

    def run(in_maps):
        return run_staged(stage(in_maps))

    class Runner:
        pass

    r = Runner()
    r.run = run
    r.stage = stage
    r.run_staged = run_staged
    r.fn = fn
    return r


# ============================================================= host prep ====

def _t_layout(a):
    """[rows, D] → [128, DC, rows] (partition-major T-layout)."""
    return np.ascontiguousarray(
        a.T.reshape(DC, 128, a.shape[0]).transpose(1, 0, 2))


def _w_halves(wt, kind):
    """[K, F, O] transposed weight → pre-tiled [K, 2, 128, F/128, O/2]."""
    Kk, F, O = wt.shape
    arr = np.ascontiguousarray(
        wt.reshape(Kk, F // 128, 128, 2, O // 2).transpose(0, 3, 2, 1, 4))
    if kind == "f8":
        return np.clip(arr * F8S, -F8MAX, F8MAX).astype(F8_NP)
    return arr.astype(BF16_NP)


def prep_stage_a(inputs, flags):
    x = np.asarray(inputs["x"], np.float32)
    rms_w = np.asarray(inputs["rms_w"], np.float32)

    def rms_nw(v):  # rms without the learned scale (folded into wproj)
        return v / np.sqrt((v * v).mean(-1, keepdims=True) + EPS_RMS)

    xr = x[:, :P, :].reshape(ROWS, D)
    mh0_full = rms_nw(xr)
    me_full = np.stack([
        rms_nw(x[:, kk + 1:kk + 1 + P, :].reshape(ROWS, D)) for kk in range(K)])

    mh0_cores, me_cores = [], []
    for c in range(NCORES):
        r0 = c * RPC
        n = max(0, min(RPC, ROWS - r0))
        sl = np.zeros((RPC, D), np.float32)
        sl[:n] = mh0_full[r0:r0 + n]
        mh0_cores.append(_t_layout(sl).astype(BF16_NP))
        mes = []
        for kk in range(K):
            sk = np.zeros((RPC, D), np.float32)
            sk[:n] = me_full[kk, r0:r0 + n]
            mes.append(_t_layout(sk))
        me_cores.append(np.stack(mes).astype(BF16_NP))

    proj_w = np.asarray(inputs["proj_w"], np.float32)
    # fold rms_w into the proj weight's input rows (exact)
    scale = np.concatenate([rms_w, rms_w])            # [2D]
    wproj = proj_w.transpose(0, 2, 1) * scale[None, :, None]
    wv = np.asarray(inputs["attn_in_w"], np.float32)[:, 2 * D:, :]
    wao = np.asarray(inputs["attn_out_w"], np.float32)
    # fold V-projection and attention out-projection: sa = p @ (wao@wv).T
    wsa = np.einsum('ked,kdf->kef', wao, wv)          # [K, D, D]

    weights = {
        "wproj": _w_halves(wproj, A_WDT["wproj"]),
        "wsa": _w_halves(wsa.transpose(0, 2, 1), A_WDT["wsa"]),
        "wf1": _w_halves(np.asarray(
            inputs["ff1_w"], np.float32).transpose(0, 2, 1), A_WDT["wf1"]),
        "wf2": _w_halves(np.asarray(
            inputs["ff2_w"], np.float32).transpose(0, 2, 1), A_WDT["wf2"]),
    }
    if flags.get("pbias"):
        weights["pbias"] = np.asarray(inputs["proj_b"], np.float32)
    if flags.get("aobias"):
        # attn_in_b's V rows flow through wao: fold into the attn_out bias
        bv = np.asarray(inputs["attn_in_b"], np.float32)[:, 2 * D:]
        weights["aobias"] = (np.asarray(inputs["attn_out_b"], np.float32)
                             + np.einsum('ked,kd->ke', wao, bv))
    if flags.get("f1bias"):
        f1b = np.asarray(inputs["ff1_b"], np.float32)
        if A_WDT["wf2"] == "f8":
            f1b = f1b / F8S  # relu consume applies scale to (ps + bias)
        weights["f1bias"] = f1b
    if flags.get("f2bias"):
        weights["f2bias"] = np.asarray(inputs["ff2_b"], np.float32)
    if flags.get("ln1"):
        weights["ln1w"] = np.asarray(inputs["ln1_w"], np.float32)
        weights["ln1b"] = np.asarray(inputs["ln1_b"], np.float32)
    if flags.get("ln2"):
        weights["ln2w"] = np.asarray(inputs["ln2_w"], np.float32)
        weights["ln2b"] = np.asarray(inputs["ln2_b"], np.float32)
    return mh0_cores, me_cores, weights


def prep_stage_b_emb(embed_w):
    """→ per-core [128, NBLK, DC, 512] bf16, contiguous per partition-row."""
    VB = NBLK * 512
    embT = np.zeros((D, NCORES * VB), dtype=BF16_NP)
    et = np.asarray(embed_w, np.float32).T.astype(BF16_NP)   # [D, V]
    for s in range(NCORES):
        n = min(VS, max(0, V - s * VS))
        embT[:, s * VB:s * VB + n] = et[:, s * VS:s * VS + n]
    shards = []
    for s in range(NCORES):
        sh = embT[:, s * VB:(s + 1) * VB]                    # [D, NBLK*512]
        shards.append(np.ascontiguousarray(
            sh.reshape(DC, 128, NBLK, 512).transpose(1, 2, 0, 3)))
    return shards


_CACHE = {}


def kernel(**inputs):
    flags = {
        "pbias": np.any(np.asarray(inputs["proj_b"])),
        "aobias": (np.any(np.asarray(inputs["attn_out_b"]))
                   or np.any(np.asarray(inputs["attn_in_b"])[:, 2 * D:])),
        "f1bias": np.any(np.asarray(inputs["ff1_b"])),
        "f2bias": np.any(np.asarray(inputs["ff2_b"])),
        "ln1": (np.any(np.asarray(inputs["ln1_b"]))
                or not np.all(np.asarray(inputs["ln1_w"]) == 1.0)),
        "ln2": (np.any(np.asarray(inputs["ln2_b"]))
                or not np.all(np.asarray(inputs["ln2_w"]) == 1.0)),
    }
    flags = {k: bool(v) for k, v in flags.items()}

    key = ("a", tuple(sorted(flags.items())))
    if key not in _CACHE:
        _CACHE[key] = make_runner(build_stage_a(flags))
    run_a = _CACHE[key]
    if "b" not in _CACHE:
        _CACHE["b"] = make_runner(build_stage_b())
    run_b = _CACHE["b"]

    mh0_cores, me_cores, weights = prep_stage_a(inputs, flags)
    in_maps_a = [dict(mh0=mh0_cores[c], me=me_cores[c], **weights)
                 for c in range(NCORES)]
    res_a = run_a.run(in_maps_a)

    # gather h: [K, DC, 128, RPC] per core → [K, D, ROWS_B] bf16
    # (rows 506..511 are stage-A pad rows; their logits are dropped later)
    hT = np.concatenate([r["hTo"].reshape(K, D, RPC) for r in res_a], axis=2)

    emb_shards = prep_stage_b_emb(inputs["embed_w"])
    in_maps_b = [{"hT": hT, "emb": emb_shards[c]} for c in range(NCORES)]
    res_b = run_b.run(in_maps_b)

    parts = []
    for r in res_b:
        ob = r["out"]                    # [NBLK, 128(rp), K, NRT(rt), 512]
        f = ob.transpose(3, 1, 2, 0, 4).reshape(NRT * 128, K, NBLK * 512)
        parts.append(f[:ROWS, :, :VS])   # drop row pad and block pad
    full = np.concatenate(parts, axis=2)             # [ROWS, K, VPAD]
    return np.ascontiguousarray(
        full[:, :, :V].astype(np.float32)).reshape(B, P, K, V)


# revision 18
# speedup vs baseline: 1.0255x; 1.0255x over previous
"""Trainium2 Bass kernel for BasicMultiTokenPrediction.

Strategy (8 NeuronCores, SPMD, two launches):
  Stage A — the 3 sequential transformer layers, data-parallel over the 506
  independent (batch, position) rows: 64 rows per core, activations kept
  transposed in SBUF ([feature-on-partition, row-on-free]).  The attention
  V-projection and out-projection are folded on the host into a single
  [D, D] matrix (softmax over one key is 1), removing one GEMM per layer.
  The sa/ff1/ff2 weights are stored in fp8-e3m4 (x128 scale, exact since
  128 = 2^7; the matching 1/128 is folded into the producing op of each
  GEMM's bf16 input so PSUM comes out at true scale); proj stays bf16.
  This cuts the replicated per-core weight DMA from 48MB to ~28MB, which
  is the stage-A bottleneck (HBM-per-core ~358 GB/s).  Weight tiles are
  split into two output-halves so MMs start when half a weight has landed.
  Stage B — the dominant vocab unembed GEMM, tensor-parallel over embed
  rows: each core takes a 6400-wide vocab shard (V padded 50257→51200) and
  all 3*506 hidden rows. lhsT = hidden states, rhs = embed shard streamed
  through SBUF; 8 accumulating matmuls per 128x512 PSUM tile.  PSUM is
  drained to bf16 staging tiles (2x DVE/ACT copy throughput, half the
  store DMA) and written to DRAM in ~1.5MB chunks; the host upcasts.

Host side: shards/transposes/casts inputs, gathers h between launches,
concatenates vocab shards at the end.  Zero biases / unit norm scales skip
their device ops entirely (general path emitted only when needed; rms_w is
always folded into the proj weights on the host, which is exact).
"""
import sys

try:
    import concourse.bass  # noqa: F401  (already on path in this env?)
except ImportError:
    sys.path.insert(0, "/opt/trn_rl_repo")

import numpy as np
import ml_dtypes
import concourse.bass as bass
import concourse.mybir as mybir
import concourse.tile as tile
from concourse import bacc
from concourse import bass2jax

BF16 = mybir.dt.bfloat16
F32 = mybir.dt.float32
F8 = mybir.dt.float8e3            # e3m4
BF16_NP = ml_dtypes.bfloat16
F8_NP = ml_dtypes.float8_e3m4
F8S = 128.0                       # fp8 weight scale (2^7, exact)
F8MAX = 15.5

B, T, D, K, V, DFF = 2, 256, 1024, 3, 50257, 2048
P = T - K            # 253 starting positions
ROWS = B * P         # 506 independent rows
NCORES = 8
VS = 6400            # per-core vocab shard (8*6400 = 51200 >= 50257)
VPAD = VS * NCORES
DC = D // 128        # 8 contraction chunks of 128
RPC = 64             # stage-A rows per core (8*64 = 512 >= 506)
FC_PROJ = (2 * D) // 128
OC = D // 128
EPS_LN = 1e-5
EPS_RMS = 1e-6

# which stage-A weights are stored fp8-e3m4 (proj must stay bf16)
A_WDT = {"wproj": "bf16", "wsa": "f8", "wf1": "f8", "wf2": "f8"}
OUT_BF16 = True      # stage-B logits staged+stored as bf16, host upcasts

ROWS_B = 512         # stage-B rows padded to 4 full 128-row tiles
NRT = ROWS_B // 128
V_BLOCKS = [(j * 512, 512) for j in range(VS // 512)]
if VS % 512:
    V_BLOCKS.append((VS - VS % 512, VS % 512))

RELU = mybir.ActivationFunctionType.Relu


# =============================================================== stage A ====

def build_stage_a(flags, repeat=1, ab_mode=None):
    """flags: dict of bools — pbias, vbias, aobias, f1bias, f2bias, ln1, ln2.
    Weights arrive pre-tiled [K, 2, 128, FC, O/2] (output-dim halves) so
    every DMA reads a fully contiguous per-partition block.
    ab_mode: None | 'dma_only' (loads only) | 'mm_only' (no weight DMA)."""
    nc = bacc.Bacc("TRN2", target_bir_lowering=False, debug=False,
                   num_devices=NCORES)
    sa_f8 = A_WDT["wsa"] == "f8"
    f1_f8 = A_WDT["wf1"] == "f8"
    f2_f8 = A_WDT["wf2"] == "f8"
    assert A_WDT["wproj"] == "bf16"
    wdt = {k: (F8 if v == "f8" else BF16) for k, v in A_WDT.items()}

    mh0 = nc.dram_tensor("mh0", [128, DC, RPC], BF16, kind="ExternalInput").ap()
    me = nc.dram_tensor("me", [K, 128, DC, RPC], BF16, kind="ExternalInput").ap()
    wproj = nc.dram_tensor("wproj", [K, 2, 128, FC_PROJ, D // 2],
                           wdt["wproj"], kind="ExternalInput").ap()
    wsa = nc.dram_tensor("wsa", [K, 2, 128, DC, D // 2], wdt["wsa"],
                         kind="ExternalInput").ap()
    wf1 = nc.dram_tensor("wf1", [K, 2, 128, DC, DFF // 2], wdt["wf1"],
                         kind="ExternalInput").ap()
    wf2 = nc.dram_tensor("wf2", [K, 2, 128, FC_PROJ, D // 2], wdt["wf2"],
                         kind="ExternalInput").ap()
    hTo = nc.dram_tensor("hTo", [K, DC, 128, RPC], BF16,
                         kind="ExternalOutput").ap()

    bias_aps = {}
    for nm, width in (("pbias", D), ("vbias", D), ("aobias", D),
                      ("f1bias", DFF), ("f2bias", D)):
        if flags.get(nm):
            bias_aps[nm] = nc.dram_tensor(
                nm, [K, width], F32, kind="ExternalInput").ap()
    for nm in ("ln1", "ln2"):
        if flags.get(nm):
            bias_aps[nm + "w"] = nc.dram_tensor(
                nm + "w", [K, D], F32, kind="ExternalInput").ap()
            bias_aps[nm + "b"] = nc.dram_tensor(
                nm + "b", [K, D], F32, kind="ExternalInput").ap()

    with tile.TileContext(nc) as tc:
        with (
            tc.tile_pool(name="singles", bufs=1) as singles,
            tc.tile_pool(name="wpool", bufs=3) as wpool,
            tc.tile_pool(name="bpool", bufs=2) as bpool,
            tc.tile_pool(name="acts", bufs=2) as acts,
            tc.tile_pool(name="stats", bufs=4) as stats,
            tc.tile_pool(name="gp", bufs=4, space="PSUM") as gp,
            tc.tile_pool(name="sp", bufs=2, space="PSUM") as sp,
            tc.tile_pool(name="bp", bufs=2, space="PSUM") as bp,
        ):
            ones128 = singles.tile([128, 1], F32, name="ones128")
            nc.vector.memset(ones128, 1.0)
            ones1 = singles.tile([1, 128], F32, name="ones1")
            nc.vector.memset(ones1, 1.0)
            eps_ln = singles.tile([1, 1], F32, name="eps_ln")
            nc.vector.memset(eps_ln, EPS_LN)
            eps_rms = singles.tile([1, 1], F32, name="eps_rms")
            nc.vector.memset(eps_rms, EPS_RMS)

            w_ring = [0]

            def load_w(wdram, k, fc, oh, nm):
                halves = []
                for hi in range(2):
                    t = wpool.tile([128, fc, oh], wdt[nm], tag=f"w_{nm}",
                                   name=f"{nm}_{k}_{hi}")
                    # alternate loads between the two HW-DGE rings so the
                    # ~2µs completion tails overlap
                    if ab_mode != "mm_only":
                        eng = nc.sync if w_ring[0] % 2 == 0 else nc.scalar
                        w_ring[0] += 1
                        eng.dma_start(out=t, in_=wdram[k, hi])
                    halves.append(t)
                return halves

            def load_bias(nm, k, width):
                t = bpool.tile([128, width // 128], F32, tag=f"b_{nm}",
                               name=f"{nm}_{k}")
                nc.sync.dma_start(out=t, in_=bias_aps[nm][k].rearrange(
                    "(c p) -> p c", p=128))
                return t

            def gemm(halves, fc, oh, rhs, consume, nm, k):
                otn = oh // 128
                for hi, Wsb in enumerate(halves):
                    for ot in range(otn):
                        ps = gp.tile([128, RPC], F32, tag="gp",
                                     name=f"ps_{nm}_{k}_{hi}_{ot}")
                        for f in range(fc):
                            nc.tensor.matmul(
                                ps, lhsT=Wsb[:, f, ot * 128:(ot + 1) * 128],
                                rhs=rhs[:, f, :], start=(f == 0),
                                stop=(f == fc - 1))
                        consume(hi * otn + ot, ps)

            def colsum(src, nm, k):
                ps = sp.tile([1, RPC], F32, tag="sp", name=f"ss_{nm}_{k}")
                for c in range(DC):
                    nc.tensor.matmul(ps, lhsT=ones128, rhs=src[:, c, :],
                                     start=(c == 0), stop=(c == DC - 1))
                return ps

            def bcast(vec, nm, k):
                ps = bp.tile([128, RPC], F32, tag="bp", name=f"bc_{nm}_{k}")
                nc.tensor.matmul(ps, lhsT=ones1, rhs=vec, start=True, stop=True)
                return ps

            def rstd_from_ex2(ex2, eps_tile, nm, k):
                sd = stats.tile([1, RPC], F32, tag="sd", name=f"sd_{nm}_{k}")
                nc.scalar.activation(out=sd, in_=ex2,
                                     func=mybir.ActivationFunctionType.Sqrt,
                                     bias=eps_tile, scale=1.0)
                rs = stats.tile([1, RPC], F32, tag="rs", name=f"rs_{nm}_{k}")
                nc.vector.reciprocal(out=rs, in_=sd)
                return rs

            def layernorm(src, outs, nm, k, affine=None):
                """outs: list of (tile [128, DC, RPC], scale); first is the
                fp32 primary (scale must be 1), extras written from it on
                the scalar engine with their scale folded in."""
                prim, pscale = outs[0]
                assert pscale == 1.0
                sq = acts.tile([128, DC, RPC], F32, tag="sq", name=f"sq_{nm}_{k}")
                for c in range(DC):
                    nc.vector.tensor_mul(out=sq[:, c, :], in0=src[:, c, :],
                                         in1=src[:, c, :])
                ps_sum = colsum(src, f"{nm}s", k)
                ps_ss = colsum(sq, f"{nm}q", k)
                m1 = stats.tile([1, RPC], F32, tag="m1", name=f"m1_{nm}_{k}")
                nc.scalar.mul(out=m1, in_=ps_sum, mul=1.0 / D)
                e1 = stats.tile([1, RPC], F32, tag="e1", name=f"e1_{nm}_{k}")
                nc.scalar.mul(out=e1, in_=ps_ss, mul=1.0 / D)
                msq = stats.tile([1, RPC], F32, tag="msq", name=f"msq_{nm}_{k}")
                nc.vector.tensor_mul(out=msq, in0=m1, in1=m1)
                var = stats.tile([1, RPC], F32, tag="var", name=f"var_{nm}_{k}")
                nc.vector.tensor_sub(out=var, in0=e1, in1=msq)
                rs = rstd_from_ex2(var, eps_ln, nm, k)
                mb_ps = bcast(m1, f"{nm}m", k)
                rb_ps = bcast(rs, f"{nm}r", k)
                mb = acts.tile([128, RPC], F32, tag="mb", name=f"mb_{nm}_{k}")
                nc.vector.tensor_copy(out=mb, in_=mb_ps)
                rb = acts.tile([128, RPC], F32, tag="rb", name=f"rb_{nm}_{k}")
                nc.vector.tensor_copy(out=rb, in_=rb_ps)
                for c in range(DC):
                    cen = sq[:, c, :]
                    nc.vector.tensor_sub(out=cen, in0=src[:, c, :], in1=mb)
                    if affine is None:
                        nc.vector.tensor_mul(out=prim[:, c, :], in0=cen, in1=rb)
                    else:
                        w_t, b_t = affine
                        nc.vector.tensor_mul(out=cen, in0=cen, in1=rb)
                        nc.vector.tensor_scalar(
                            out=prim[:, c, :], in0=cen,
                            scalar1=w_t[:, c:c + 1], scalar2=b_t[:, c:c + 1],
                            op0=mybir.AluOpType.mult, op1=mybir.AluOpType.add)
                    for out_t, s in outs[1:]:
                        if s == 1.0:
                            nc.scalar.copy(out=out_t[:, c, :],
                                           in_=prim[:, c, :])
                        else:
                            nc.scalar.mul(out=out_t[:, c, :],
                                          in_=prim[:, c, :], mul=s)

            def ln_center(src, outs, nm, k):
                """Subtract the per-row feature mean only (no rstd).  Exact
                when ln1's affine is identity: the skipped per-row scale is
                positive and cancels through relu/ff GEMMs into ln2, which
                is scale-invariant."""
                prim, pscale = outs[0]
                assert pscale == 1.0
                ps_sum = colsum(src, f"{nm}s", k)
                m1 = stats.tile([1, RPC], F32, tag="m1", name=f"m1_{nm}_{k}")
                nc.scalar.mul(out=m1, in_=ps_sum, mul=1.0 / D)
                mb_ps = bcast(m1, f"{nm}m", k)
                mb = acts.tile([128, RPC], F32, tag="mb", name=f"mb_{nm}_{k}")
                nc.vector.tensor_copy(out=mb, in_=mb_ps)
                for c in range(DC):
                    nc.vector.tensor_sub(out=prim[:, c, :], in0=src[:, c, :],
                                         in1=mb)
                    for out_t, s in outs[1:]:
                        if s == 1.0:
                            nc.scalar.copy(out=out_t[:, c, :],
                                           in_=prim[:, c, :])
                        else:
                            nc.scalar.mul(out=out_t[:, c, :],
                                          in_=prim[:, c, :], mul=s)

            # mrg kept as two tiles: the e-half arrives by DMA and has no
            # dependency on the previous layer, so its proj MMs can fill the
            # PE while the h-half's rms/ln2 stat chains are in flight.
            mrgh0 = acts.tile([128, DC, RPC], BF16, tag="mrgh", name="mrgh_0")
            mrge0 = acts.tile([128, DC, RPC], BF16, tag="mrge", name="mrge_0")
            # ACT ring: keep the small input loads out of the SP ring's FIFO
            # so the first proj weight half starts streaming immediately
            nc.scalar.dma_start(out=mrgh0, in_=mh0)
            nc.scalar.dma_start(out=mrge0, in_=me[0])

            def a_body(iv=None):
                mrgh_k, mrge_k = mrgh0, mrge0
                for k in range(K):
                    Wp = load_w(wproj, k, FC_PROJ, D // 2, "wproj")
                    Ws = load_w(wsa, k, DC, D // 2, "wsa")
                    W1 = load_w(wf1, k, DC, DFF // 2, "wf1")
                    W2 = load_w(wf2, k, FC_PROJ, D // 2, "wf2")
                    if ab_mode == "dma_only":
                        continue
                    pb = load_bias("pbias", k, D) if flags.get("pbias") else None
                    ab = load_bias("aobias", k, D) if flags.get("aobias") else None
                    f1b = load_bias("f1bias", k, DFF) if flags.get("f1bias") else None
                    f2b = load_bias("f2bias", k, D) if flags.get("f2bias") else None
                    ln1a = ((load_bias("ln1w", k, D), load_bias("ln1b", k, D))
                            if flags.get("ln1") else None)
                    ln2a = ((load_bias("ln2w", k, D), load_bias("ln2b", k, D))
                            if flags.get("ln2") else None)

                    p_f32 = acts.tile([128, DC, RPC], F32, tag="p_f32",
                                      name=f"p_f32_{k}")
                    p_bf = acts.tile([128, DC, RPC], BF16, tag="p_bf",
                                     name=f"p_bf_{k}")

                    def c_proj(ot, ps, p_f32=p_f32, p_bf=p_bf, pb=pb):
                        if pb is None:
                            nc.vector.tensor_copy(out=p_f32[:, ot, :], in_=ps)
                        else:
                            nc.vector.tensor_scalar_add(
                                out=p_f32[:, ot, :], in0=ps,
                                scalar1=pb[:, ot:ot + 1])
                        if sa_f8:
                            nc.scalar.mul(out=p_bf[:, ot, :],
                                          in_=p_f32[:, ot, :], mul=1.0 / F8S)
                        else:
                            nc.vector.tensor_copy(out=p_bf[:, ot, :],
                                                  in_=p_f32[:, ot, :])

                    otn = (D // 2) // 128
                    for hi, Wsb in enumerate(Wp):
                        for ot in range(otn):
                            ps = gp.tile([128, RPC], F32, tag="gp",
                                         name=f"ps_proj_{k}_{hi}_{ot}")
                            for f in range(FC_PROJ):
                                rhs = (mrgh_k[:, f, :] if f < DC
                                       else mrge_k[:, f - DC, :])
                                nc.tensor.matmul(
                                    ps, lhsT=Wsb[:, f, ot * 128:(ot + 1) * 128],
                                    rhs=rhs, start=(f == 0),
                                    stop=(f == FC_PROJ - 1))
                            c_proj(hi * otn + ot, ps)

                    r1 = acts.tile([128, DC, RPC], F32, tag="r1", name=f"r1_{k}")

                    def c_sa(ot, ps, r1=r1, p_f32=p_f32, ab=ab):
                        nc.vector.tensor_add(out=r1[:, ot, :], in0=ps,
                                             in1=p_f32[:, ot, :])
                        if ab is not None:
                            nc.vector.tensor_scalar_add(
                                out=r1[:, ot, :], in0=r1[:, ot, :],
                                scalar1=ab[:, ot:ot + 1])
                    gemm(Ws, DC, D // 2, p_bf, c_sa, "sa", k)

                    s1_f32 = acts.tile([128, DC, RPC], F32, tag="s1_f32",
                                       name=f"s1_f32_{k}")
                    s1_bf = acts.tile([128, DC, RPC], BF16, tag="s1_bf",
                                      name=f"s1_bf_{k}")
                    s1_outs = [(s1_f32, 1.0),
                               (s1_bf, (1.0 / F8S) if f1_f8 else 1.0)]
                    if ln1a is None:
                        ln_center(r1, s1_outs, "ln1", k)
                    else:
                        layernorm(r1, s1_outs, "ln1", k, affine=ln1a)

                    ff_bf = acts.tile([128, FC_PROJ, RPC], BF16, tag="ff_bf",
                                      name=f"ff_bf_{k}")
                    relu_scale = (1.0 / F8S) if f2_f8 else 1.0

                    def c_ff1(ot, ps, ff_bf=ff_bf, f1b=f1b, rs=relu_scale):
                        if f1b is None and rs == 1.0:
                            nc.vector.tensor_relu(out=ff_bf[:, ot, :], in_=ps)
                        else:
                            # out = relu(ps*rs + f1b*rs); host pre-scales f1b
                            nc.scalar.activation(
                                out=ff_bf[:, ot, :], in_=ps, func=RELU,
                                bias=(0.0 if f1b is None
                                      else f1b[:, ot:ot + 1]),
                                scale=rs)
                    gemm(W1, DC, DFF // 2, s1_bf, c_ff1, "ff1", k)

                    r2 = acts.tile([128, DC, RPC], F32, tag="r2", name=f"r2_{k}")

                    def c_ff2(ot, ps, r2=r2, s1_f32=s1_f32, f2b=f2b):
                        nc.vector.tensor_add(out=r2[:, ot, :], in0=ps,
                                             in1=s1_f32[:, ot, :])
                        if f2b is not None:
                            nc.vector.tensor_scalar_add(
                                out=r2[:, ot, :], in0=r2[:, ot, :],
                                scalar1=f2b[:, ot:ot + 1])
                    gemm(W2, FC_PROJ, D // 2, ff_bf, c_ff2, "ff2", k)

                    h_bf = acts.tile([128, DC, RPC], BF16, tag="h_bf",
                                     name=f"h_bf_{k}")
                    layernorm(r2, [(h_bf, 1.0)], "ln2", k, affine=ln2a)

                    nc.sync.dma_start(
                        out=hTo[k].rearrange("c p r -> p c r"), in_=h_bf)

                    if k < K - 1:
                        mrgh_k = acts.tile([128, DC, RPC], BF16, tag="mrgh",
                                           name=f"mrgh_{k + 1}")
                        mrge_k = acts.tile([128, DC, RPC], BF16, tag="mrge",
                                           name=f"mrge_{k + 1}")
                        nc.sync.dma_start(out=mrge_k, in_=me[k + 1])
                        sqh = acts.tile([128, DC, RPC], F32, tag="sq",
                                        name=f"sqh_{k}")
                        for c in range(DC):
                            nc.vector.tensor_mul(out=sqh[:, c, :],
                                                 in0=h_bf[:, c, :],
                                                 in1=h_bf[:, c, :])
                        ps_ss = colsum(sqh, "rms", k)
                        e1 = stats.tile([1, RPC], F32, tag="e1",
                                        name=f"e1_rms_{k}")
                        nc.scalar.mul(out=e1, in_=ps_ss, mul=1.0 / D)
                        rs = rstd_from_ex2(e1, eps_rms, "rms", k)
                        rb_ps = bcast(rs, "rms", k)
                        rb = acts.tile([128, RPC], F32, tag="rb",
                                       name=f"rb_rms_{k}")
                        nc.vector.tensor_copy(out=rb, in_=rb_ps)
                        for c in range(DC):
                            nc.vector.tensor_mul(out=mrgh_k[:, c, :],
                                                 in0=h_bf[:, c, :], in1=rb)

            if repeat > 1:
                with tc.For_i(0, repeat, 1):
                    a_body()
            else:
                a_body()
    nc.compile()
    return nc


# =============================================================== stage B ====

NBLK = len(V_BLOCKS)
OUT_DT = BF16 if OUT_BF16 else F32
OUT_DT_NP = BF16_NP if OUT_BF16 else np.float32


def build_stage_b(repeat=1, mm_only=False, no_dma=False, out_split=True,
                  psum_bufs=2, spool_bufs=3, copy_eng="split",
                  ring_swap=False):
    """Output is written block-major ([NBLK, 128, K, NRT, 512]) so every
    store is one fully-contiguous DMA; the host untangles the layout.
    PSUM tiles span 4 banks (all 4 row-tiles of one k) so bank-handoff
    semaphores and drain copies happen 4x less often."""
    nc = bacc.Bacc("TRN2", target_bir_lowering=False, debug=False,
                   num_devices=NCORES)
    hT = nc.dram_tensor("hT", [K, D, ROWS_B], BF16, kind="ExternalInput").ap()
    emb = nc.dram_tensor("emb", [128, NBLK, DC, 512], BF16,
                         kind="ExternalInput").ap()
    out = nc.dram_tensor("out", [NBLK, 128, K, NRT, 512], OUT_DT,
                         kind="ExternalOutput").ap()

    with tile.TileContext(nc) as tc:
        with (
            tc.tile_pool(name="hpool", bufs=1) as hpool,
            tc.tile_pool(name="epool", bufs=4) as epool,
            tc.tile_pool(name="spool", bufs=spool_bufs) as spool,
            tc.tile_pool(name="psum", bufs=psum_bufs, space="PSUM") as pp,
        ):
            hsb = []
            for k in range(K):
                row = []
                for c in range(DC):
                    t = hpool.tile([128, ROWS_B], BF16, name=f"h_{k}_{c}")
                    # ACT ring: the embed stream owns the SP ring, so the h
                    # preload doesn't delay the first v-block
                    nc.scalar.dma_start(out=t,
                                        in_=hT[k, c * 128:(c + 1) * 128, :])
                    row.append(t)
                hsb.append(row)

            def b_body(iv=None):
                for j, (v0, vn) in enumerate(V_BLOCKS):
                    ej = epool.tile([128, DC, 512], BF16, name="ej", tag="ej")
                    (nc.scalar if ring_swap else nc.sync).dma_start(
                        out=ej, in_=emb[:, j])
                    stb = spool.tile([128, K, NRT, 512], OUT_DT, name="stb",
                                     tag="st")
                    for k in range(K):
                        ps = pp.tile([128, NRT, 512], F32, name="ps", tag="ps")
                        for rt in range(NRT):
                            for c in range(DC):
                                nc.tensor.matmul(
                                    ps[:, rt, :vn],
                                    lhsT=hsb[k][c][:, rt * 128:(rt + 1) * 128],
                                    rhs=ej[:, c, :vn],
                                    start=(c == 0), stop=(c == DC - 1))
                        if not mm_only:
                            # one 4-bank drain per k; alternate DVE/ACT
                            use_dve = (copy_eng == "dve" or
                                       (copy_eng == "split" and k % 2 == 0))
                            if use_dve:
                                nc.vector.tensor_copy(
                                    out=stb[:, k, :, :vn],
                                    in_=ps[:, :, :vn])
                            else:
                                nc.scalar.copy(out=stb[:, k, :, :vn],
                                               in_=ps[:, :, :vn])
                    if mm_only or no_dma:
                        continue
                    if ring_swap:
                        eng = nc.sync
                    else:
                        eng = nc.sync if (out_split and j % 2) else nc.scalar
                    eng.dma_start(out=out[j], in_=stb)

            if repeat > 1:
                with tc.For_i(0, repeat, 1):
                    b_body()
            else:
                b_body()
    nc.compile()
    return nc


# ================================================================ runner ====

def make_runner(nc, n_cores=NCORES):
    import jax
    from jax.sharding import Mesh, PartitionSpec
    from jax.experimental.shard_map import shard_map

    bass2jax.install_neuronx_cc_hook()
    partition_name = nc.partition_id_tensor.name if nc.partition_id_tensor else None
    in_names, out_names, out_avals, zero_outs = [], [], [], []
    for alloc in nc.m.functions[0].allocations:
        if not isinstance(alloc, mybir.MemoryLocationSet):
            continue
        name = alloc.memorylocations[0].name
        if alloc.kind == "ExternalInput":
            if name != partition_name:
                in_names.append(name)
        elif alloc.kind == "ExternalOutput":
            out_names.append(name)
            shape = tuple(alloc.tensor_shape)
            dtype = mybir.dt.np(alloc.dtype)
            out_avals.append(jax.core.ShapedArray(shape, dtype))
            zero_outs.append(np.zeros(shape, dtype))
    n_params = len(in_names)
    in_names_all = in_names + out_names
    if partition_name is not None:
        in_names_all.append(partition_name)

    def _body(*args):
        operands = list(args)
        if partition_name is not None:
            operands.append(bass2jax.partition_id_tensor())
        outs = bass2jax._bass_exec_p.bind(
            *operands, out_avals=tuple(out_avals), in_names=tuple(in_names_all),
            out_names=tuple(out_names), lowering_input_output_aliases=(),
            sim_require_finite=True, sim_require_nnan=True, nc=nc)
        return tuple(outs)

    devices = jax.devices()[:n_cores]
    mesh = Mesh(np.asarray(devices), ("core",))
    in_specs = (PartitionSpec("core"),) * (n_params + len(out_names))
    out_specs = (PartitionSpec("core"),) * len(out_names)
    fn = jax.jit(
        shard_map(_body, mesh=mesh, in_specs=in_specs, out_specs=out_specs,
                  check_rep=False),
        keep_unused=True)

    def stage(in_maps):
        from jax.sharding import NamedSharding
        sh = NamedSharding(mesh, PartitionSpec("core"))
        concat_in = [np.concatenate([np.asarray(m[name]) for m in in_maps],
                                    axis=0) for name in in_names]
        concat_zeros = [np.zeros((n_cores * z.shape[0], *z.shape[1:]), z.dtype)
                        for z in zero_outs]
        return [jax.device_put(a, sh) for a in concat_in + concat_zeros]

    def run_staged(args):
        out_arrs = fn(*args)
        return [
            {name: np.asarray(out_arrs[i]).reshape(n_cores, *out_avals[i].shape)[c]
             for i, name in enumerate(out_names)}
            for c in range(n_cores)
        ]

    def run(in_maps):
        return run_staged(stage(in_maps))

    class Runner:
        pass

    r = Runner()
    r.run = run
    r.stage = stage
    r.run_staged = run_staged
    r.fn = fn
    return r


# ============================================================= host prep ====

def _t_layout(a):
    """[rows, D] → [128, DC, rows] (partition-major T-layout)."""
    return np.ascontiguousarray(
        a.T.reshape(DC, 128, a.shape[0]).transpose(1, 0, 2))


def _w_halves(wt, kind):
    """[K, F, O] transposed weight → pre-tiled [K, 2, 128, F/128, O/2]."""
    Kk, F, O = wt.shape
    arr = np.ascontiguousarray(
        wt.reshape(Kk, F // 128, 128, 2, O // 2).transpose(0, 3, 2, 1, 4))
    if kind == "f8":
        return np.clip(arr * F8S, -F8MAX, F8MAX).astype(F8_NP)
    return arr.astype(BF16_NP)


def prep_stage_a(inputs, flags):
    x = np.asarray(inputs["x"], np.float32)
    rms_w = np.asarray(inputs["rms_w"], np.float32)

    def rms_nw(v):  # rms without the learned scale (folded into wproj)
        return v / np.sqrt((v * v).mean(-1, keepdims=True) + EPS_RMS)

    xr = x[:, :P, :].reshape(ROWS, D)
    mh0_full = rms_nw(xr)
    me_full = np.stack([
        rms_nw(x[:, kk + 1:kk + 1 + P, :].reshape(ROWS, D)) for kk in range(K)])

    mh0_cores, me_cores = [], []
    for c in range(NCORES):
        r0 = c * RPC
        n = max(0, min(RPC, ROWS - r0))
        sl = np.zeros((RPC, D), np.float32)
        sl[:n] = mh0_full[r0:r0 + n]
        mh0_cores.append(_t_layout(sl).astype(BF16_NP))
        mes = []
        for kk in range(K):
            sk = np.zeros((RPC, D), np.float32)
            sk[:n] = me_full[kk, r0:r0 + n]
            mes.append(_t_layout(sk))
        me_cores.append(np.stack(mes).astype(BF16_NP))

    proj_w = np.asarray(inputs["proj_w"], np.float32)
    # fold rms_w into the proj weight's input rows (exact)
    scale = np.concatenate([rms_w, rms_w])            # [2D]
    wproj = proj_w.transpose(0, 2, 1) * scale[None, :, None]
    wv = np.asarray(inputs["attn_in_w"], np.float32)[:, 2 * D:, :]
    wao = np.asarray(inputs["attn_out_w"], np.float32)
    # fold V-projection and attention out-projection: sa = p @ (wao@wv).T
    wsa = np.einsum('ked,kdf->kef', wao, wv)          # [K, D, D]

    weights = {
        "wproj": _w_halves(wproj, A_WDT["wproj"]),
        "wsa": _w_halves(wsa.transpose(0, 2, 1), A_WDT["wsa"]),
        "wf1": _w_halves(np.asarray(
            inputs["ff1_w"], np.float32).transpose(0, 2, 1), A_WDT["wf1"]),
        "wf2": _w_halves(np.asarray(
            inputs["ff2_w"], np.float32).transpose(0, 2, 1), A_WDT["wf2"]),
    }
    if flags.get("pbias"):
        weights["pbias"] = np.asarray(inputs["proj_b"], np.float32)
    if flags.get("aobias"):
        # attn_in_b's V rows flow through wao: fold into the attn_out bias
        bv = np.asarray(inputs["attn_in_b"], np.float32)[:, 2 * D:]
        weights["aobias"] = (np.asarray(inputs["attn_out_b"], np.float32)
                             + np.einsum('ked,kd->ke', wao, bv))
    if flags.get("f1bias"):
        f1b = np.asarray(inputs["ff1_b"], np.float32)
        if A_WDT["wf2"] == "f8":
            f1b = f1b / F8S  # relu consume applies scale to (ps + bias)
        weights["f1bias"] = f1b
    if flags.get("f2bias"):
        weights["f2bias"] = np.asarray(inputs["ff2_b"], np.float32)
    if flags.get("ln1"):
        weights["ln1w"] = np.asarray(inputs["ln1_w"], np.float32)
        weights["ln1b"] = np.asarray(inputs["ln1_b"], np.float32)
    if flags.get("ln2"):
        weights["ln2w"] = np.asarray(inputs["ln2_w"], np.float32)
        weights["ln2b"] = np.asarray(inputs["ln2_b"], np.float32)
    return mh0_cores, me_cores, weights


def prep_stage_b_emb(embed_w):
    """→ per-core [128, NBLK, DC, 512] bf16, contiguous per partition-row."""
    VB = NBLK * 512
    embT = np.zeros((D, NCORES * VB), dtype=BF16_NP)
    et = np.asarray(embed_w, np.float32).T.astype(BF16_NP)   # [D, V]
    for s in range(NCORES):
        n = min(VS, max(0, V - s * VS))
        embT[:, s * VB:s * VB + n] = et[:, s * VS:s * VS + n]
    shards = []
    for s in range(NCORES):
        sh = embT[:, s * VB:(s + 1) * VB]                    # [D, NBLK*512]
        shards.append(np.ascontiguousarray(
            sh.reshape(DC, 128, NBLK, 512).transpose(1, 2, 0, 3)))
    return shards


_CACHE = {}


def kernel(**inputs):
    flags = {
        "pbias": np.any(np.asarray(inputs["proj_b"])),
        "aobias": (np.any(np.asarray(inputs["attn_out_b"]))
                   or np.any(np.asarray(inputs["attn_in_b"])[:, 2 * D:])),
        "f1bias": np.any(np.asarray(inputs["ff1_b"])),
        "f2bias": np.any(np.asarray(inputs["ff2_b"])),
        "ln1": (np.any(np.asarray(inputs["ln1_b"]))
                or not np.all(np.asarray(inputs["ln1_w"]) == 1.0)),
        "ln2": (np.any(np.asarray(inputs["ln2_b"]))
                or not np.all(np.asarray(inputs["ln2_w"]) == 1.0)),
    }
    flags = {k: bool(v) for k, v in flags.items()}

    key = ("a", tuple(sorted(flags.items())))
    if key not in _CACHE:
        _CACHE[key] = make_runner(build_stage_a(flags))
    run_a = _CACHE[key]
    if "b" not in _CACHE:
        _CACHE["b"] = make_runner(build_stage_b())
    run_b = _CACHE["b"]

    mh0_cores, me_cores, weights = prep_stage_a(inputs, flags)
    in_maps_a = [dict(mh0=mh0_cores[c], me=me_cores[c], **weights)
                 for c in range(NCORES)]
    res_a = run_a.run(in_maps_a)

    # gather h: [K, DC, 128, RPC] per core → [K, D, ROWS_B] bf16
    # (rows 506..511 are stage-A pad rows; their logits are dropped later)
    hT = np.concatenate([r["hTo"].reshape(K, D, RPC) for r in res_a], axis=2)

    emb_shards = prep_stage_b_emb(inputs["embed_w"])
    in_maps_b = [{"hT": hT, "emb": emb_shards[c]} for c in range(NCORES)]
    res_b = run_b.run(in_maps_b)

    parts = []
    for r in res_b:
        ob = r["out"]                    # [NBLK, 128(rp), K, NRT(rt), 512]
        f = ob.transpose(3, 1, 2, 0, 4).reshape(NRT * 128, K, NBLK * 512)
        parts.append(f[:ROWS, :, :VS])   # drop row pad and block pad
    full = np.concatenate(parts, axis=2)             # [ROWS, K, VPAD]
    return np.ascontiguousarray(
        full[:, :, :V].astype(np.float32)).reshape(B, P, K, V)
